# revision 9
# baseline (speedup 1.0000x reference)
"""Trainium2 Bass kernel for nn_ExternalLinear (noisy linear layer).

Computes: y = x @ W.T + b; power = mean(y^2); out = y + noise * sqrt(power/1000)
where noise = jax.random.normal(key(42), y.shape) (deterministic, reproduced on host).

Sharding: data-parallel over the batch axis across 8 NeuronCores. Weight/bias
replicated. One AllReduce of per-shard per-partition sum(y^2) partials.

Per-core dataflow (32768 rows):
  phase 1 (per 512-row group): DMA x -> PE transpose (x.T) -> ACT copy PSUM->SBUF
    -> 4x f32 matmul (stationary x.T tile, moving W.T) -> DVE y = psum + bias
    (y kept resident in SBUF, 128KB/partition) -> ACT Square w/ accum_out column.
  AllReduce [128,1] partials -> ones-matmul (reduce+broadcast) -> ACT sqrt -> s.
  phase 2 (per group): DMA noise -> DVE out = noise*s + y -> DMA out.
"""

import sys

import numpy as np

sys.path.insert(0, "/opt/trn_rl_repo")

B, IN, OUT = 262144, 128, 128
NCORES = 8
RPC = B // NCORES            # rows per core: 32768
TPG = 4                      # 128-row tiles per group
GROUP = 128 * TPG            # rows per group: 512
G = RPC // GROUP             # groups per core: 64
SNR_DB = 30.0
N_TOTAL = float(B * OUT)     # elements of y
POWER_SCALE = 1.0 / (N_TOTAL * (10.0 ** (SNR_DB / 10.0)))

_cache = {}


def _build_nc(repeat=1):
    import concourse.bacc as bacc
    import concourse.mybir as mybir
    from concourse import masks
    from concourse.tile import TileContext

    f32 = mybir.dt.float32
    nc = bacc.Bacc("TRN2", target_bir_lowering=False, debug=False,
                   num_devices=NCORES)

    x_d = nc.dram_tensor("x", [RPC, IN], f32, kind="ExternalInput")
    noise_d = nc.dram_tensor("noise", [RPC, OUT], f32, kind="ExternalInput")
    wt_d = nc.dram_tensor("wt", [IN, OUT], f32, kind="ExternalInput")
    biasf_d = nc.dram_tensor("biasf", [128, GROUP], f32, kind="ExternalInput")
    out_d = nc.dram_tensor("out", [RPC, OUT], f32, kind="ExternalOutput")

    xv = x_d.rearrange("(g t p) k -> g p t k", t=TPG, p=128)
    nv = noise_d.rearrange("(g t p) k -> g p t k", t=TPG, p=128)
    ov = out_d.rearrange("(g t p) k -> g p t k", t=TPG, p=128)

    with TileContext(nc) as tc:
        with (
            tc.tile_pool(name="const", bufs=1) as const,
            tc.tile_pool(name="ybig", bufs=1) as ybig,
            tc.tile_pool(name="xg", bufs=3) as xpool,
            tc.tile_pool(name="xtp", bufs=2, space="PSUM") as xtpsum,
            tc.tile_pool(name="xts", bufs=3) as xtpool,
            tc.tile_pool(name="yp", bufs=2, space="PSUM") as ypsum,
            tc.tile_pool(name="sq", bufs=2) as sqpool,
            tc.tile_pool(name="ns", bufs=4) as nspool,
            tc.tile_pool(name="og", bufs=3) as ogpool,
            tc.tile_pool(name="totp", bufs=1, space="PSUM") as totpsum,
            tc.tile_pool(name="dram", bufs=1, space="DRAM") as dram,
        ):
            wt_s = const.tile([128, OUT], f32)
            biasf_s = const.tile([128, GROUP], f32)
            ident = const.tile([128, 128], f32)
            ones = const.tile([128, 128], f32)
            acc_all = const.tile([128, G], f32)
            partial = const.tile([128, 1], f32)
            partial_ar = const.tile([128, 1], f32)
            s_bcast = const.tile([128, 1], f32)
            y_all = ybig.tile([128, G * GROUP], f32)

            nc.sync.dma_start(out=wt_s[:], in_=wt_d[:])
            nc.sync.dma_start(out=biasf_s[:], in_=biasf_d[:])
            masks.make_identity(nc, ident[:])
            nc.vector.memset(ones[:], 1.0)

            # repeat>1 builds a self-timing variant (R identical passes) for
            # slope-based exec-time measurement; repeat=1 is the real kernel.
            for _r in range(repeat):
                # -- phase 1: y = x @ W.T + b, per-partition sumsq partials --
                for g in range(G):
                    xg = xpool.tile([128, TPG, IN], f32)
                    nc.sync.dma_start(out=xg[:], in_=xv[g])
                    xt_p = xtpsum.tile([128, GROUP], f32)
                    for t in range(TPG):
                        nc.tensor.transpose(
                            xt_p[:, t * 128:(t + 1) * 128], xg[:, t, :], ident[:]
                        )
                    xt_s = xtpool.tile([128, GROUP], f32)
                    nc.scalar.copy(xt_s[:], xt_p[:])
                    y_p = ypsum.tile([128, GROUP], f32)
                    for t in range(TPG):
                        nc.tensor.matmul(
                            y_p[:, t * 128:(t + 1) * 128],
                            xt_s[:, t * 128:(t + 1) * 128],
                            wt_s[:],
                            start=True,
                            stop=True,
                        )
                    ysl = y_all[:, g * GROUP:(g + 1) * GROUP]
                    nc.vector.tensor_add(ysl, y_p[:], biasf_s[:])
                    sq = sqpool.tile([128, GROUP], f32)
                    nc.scalar.activation(
                        sq[:], ysl, mybir.ActivationFunctionType.Square,
                        accum_out=acc_all[:, g:g + 1],
                    )

                # -- global scalar: AllReduce, s = sqrt(sum/(N*1000)) --
                nc.vector.tensor_reduce(
                    partial[:], acc_all[:], axis=mybir.AxisListType.X,
                    op=mybir.AluOpType.add,
                )
                cc_in = dram.tile([128, 1], f32)
                cc_out = dram.tile([128, 1], f32)
                nc.sync.dma_start(out=cc_in[:], in_=partial[:])
                nc.gpsimd.collective_compute(
                    "AllReduce",
                    mybir.AluOpType.add,
                    replica_groups=[list(range(NCORES))],
                    ins=[cc_in.opt()],
                    outs=[cc_out.opt()],
                )
                nc.sync.dma_start(out=partial_ar[:], in_=cc_out[:])
                tot_p = totpsum.tile([128, 1], f32)
                nc.tensor.matmul(tot_p[:], ones[:], partial_ar[:],
                                 start=True, stop=True)
                nc.scalar.activation(
                    s_bcast[:], tot_p[:], mybir.ActivationFunctionType.Sqrt,
                    scale=POWER_SCALE,
                )

                # -- phase 2: out = noise * s + y --
                for g in range(G):
                    ns = nspool.tile([128, TPG, OUT], f32)
                    nc.sync.dma_start(out=ns[:], in_=nv[g])
                    og = ogpool.tile([128, TPG, OUT], f32)
                    nc.vector.scalar_tensor_tensor(
                        og[:],
                        ns[:],
                        s_bcast[:, 0:1],
                        y_all[:, g * GROUP:(g + 1) * GROUP],
                        op0=mybir.AluOpType.mult,
                        op1=mybir.AluOpType.add,
                    )
                    nc.sync.dma_start(out=ov[g], in_=og[:])

    nc.finalize()
    return nc


def _get_noise():
    import jax
    import jax.numpy as jnp

    with jax.default_device(jax.devices("cpu")[0]):
        return np.asarray(
            jax.random.normal(jax.random.key(42), (B, OUT), dtype=jnp.float32)
        )


def kernel(x, weight, bias):
    from concourse.bass_utils import run_bass_kernel_spmd

    x = np.ascontiguousarray(np.asarray(x, dtype=np.float32))
    weight = np.asarray(weight, dtype=np.float32)
    bias = np.asarray(bias, dtype=np.float32)

    if "nc" not in _cache:
        _cache["nc"] = _build_nc()
        _cache["noise"] = _get_noise()
    nc = _cache["nc"]
    noise = _cache["noise"]

    wt = np.ascontiguousarray(weight.T)                      # [k, o]
    biasf = np.ascontiguousarray(
        np.broadcast_to(np.tile(bias, TPG)[None, :], (128, GROUP))
    )

    in_maps = []
    for i in range(NCORES):
        sl = slice(i * RPC, (i + 1) * RPC)
        in_maps.append({
            "x": x[sl],
            "noise": np.ascontiguousarray(noise[sl]),
            "wt": wt,
            "biasf": biasf,
        })

    res = run_bass_kernel_spmd(nc, in_maps, list(range(NCORES)))
    out = np.concatenate([res.results[i]["out"] for i in range(NCORES)], axis=0)
    return out
